# revision 8
# baseline (speedup 1.0000x reference)
"""DeepSet (phi -> segment_sum -> rho) Bass kernel for 8 trn2 NeuronCores.

Sharding (per hint): data-parallel over segments. 16384 segments -> 8 cores x
2048 (segment-aligned row ranges via host searchsorted on sorted segment_ids).

Per-core dataflow (all host-marshalled; T 128-row tiles per 128-seg window):
  - L1 (fp32r, wide): z1[65, 512] = W1a[8, 65].T @ xT[8, 512]; feature 64 is a
    constant-one (zero weights, bias 1) providing L2's bias via contraction.
  - relu1 on ACT with per-partition bias -> h1a[65, 512] fp32.
  - L2 (fp32): h2[128rows, 64] = h1a_tile[65, 128].T @ W2a[65, 64] per tile -
    rows land on partitions, exactly what pooling needs; no transpose.
  - relu2 on DVE (max with 0) -> h2t[128, 256] per 4-tile group.
  - onehot[128rows, 128segs] per tile = (idr == iota), one batched DVE
    is_equal per group; padded rows have idr = -1 and match nothing.
  - pool (fp32): pooled[128segs, 64] += onehot[128, 128].T @ h2t[:, 64]
    PSUM-accumulated over the window's T tiles (matmul cost ~ out free size,
    so segments-on-partitions orientation is 2x cheaper).
  - per window: PE-transpose pooled -> [64, 128segs], copy into a [65, 512]
    4-window chunk; row 64 = per-segment counts (host bincount, DMA'd).
  - phi L3 commutes past the pooling (linear): batched tail per 512-seg
    chunk (fp32r, wide): L3 + counts*b3, then rho; out [4, 2048] per core.
Host gathers 8x[4, 2048] -> [16384, 4].
"""

import sys

import numpy as np

sys.path.insert(0, "/opt/trn_rl_repo")

import concourse.bass as bass  # noqa: E402
import concourse.mybir as mybir  # noqa: E402
import concourse.tile as tile  # noqa: E402
from concourse import bacc  # noqa: E402
from concourse.bass_utils import run_bass_kernel_spmd  # noqa: E402
from concourse.masks import make_identity  # noqa: E402

F32 = mybir.dt.float32
F32R = mybir.dt.float32r
I32 = mybir.dt.int32
AF = mybir.ActivationFunctionType

NUM_SEGMENTS = 16384
N_CORES = 8
SEG_PER_CORE = NUM_SEGMENTS // N_CORES  # 2048
WIN_SEGS = 128
N_WIN = SEG_PER_CORE // WIN_SEGS  # 16
STATE_DIM = 8
HID = 64
OUT_DIM = 4
GRP = 4  # tiles per op-batch group (512 rows)
CHUNK = 512  # segs per batched rho-tail chunk (4 windows)

_BUILD_CACHE: dict[tuple[int, int], object] = {}


def _build_program(T: int, reps: int = 1):
    if (T, reps) in _BUILD_CACHE:
        return _BUILD_CACHE[(T, reps)]
    assert T % GRP == 0
    PW = T * 128
    NG = T // GRP

    nc = bacc.Bacc("TRN2", target_bir_lowering=False, debug=False, num_devices=N_CORES)

    xT_d = nc.declare_dram_parameter("xT", [STATE_DIM, N_WIN * PW], F32R, isOutput=False)
    idr_d = nc.declare_dram_parameter("idr", [128, N_WIN * T], F32, isOutput=False)
    cnt_d = nc.declare_dram_parameter("cnt", [1, SEG_PER_CORE], F32R, isOutput=False)
    w1a_d = nc.declare_dram_parameter("w1a", [STATE_DIM, HID + 1], F32R, isOutput=False)
    w2a_d = nc.declare_dram_parameter("w2a", [HID + 1, HID], F32, isOutput=False)
    w3a_d = nc.declare_dram_parameter("w3a", [HID + 1, HID], F32R, isOutput=False)
    rw1_d = nc.declare_dram_parameter("rw1", [HID, HID], F32R, isOutput=False)
    rw2_d = nc.declare_dram_parameter("rw2", [HID, HID], F32R, isOutput=False)
    rw3_d = nc.declare_dram_parameter("rw3", [HID, OUT_DIM], F32R, isOutput=False)
    pb1a_d = nc.declare_dram_parameter("pb1a", [HID + 1, 1], F32, isOutput=False)
    rb1_d = nc.declare_dram_parameter("rb1", [HID, 1], F32, isOutput=False)
    rb2_d = nc.declare_dram_parameter("rb2", [HID, 1], F32, isOutput=False)
    rb3_d = nc.declare_dram_parameter("rb3", [OUT_DIM, 1], F32, isOutput=False)
    out_d = nc.declare_dram_parameter("out", [OUT_DIM, SEG_PER_CORE], F32, isOutput=True)

    with tile.TileContext(nc) as tc:
        with (
            tc.tile_pool(name="const", bufs=1) as cpool,
            tc.tile_pool(name="xwin", bufs=2) as xpool,
            tc.tile_pool(name="work", bufs=3) as wpool,
            tc.tile_pool(name="chunk", bufs=2) as chpool,
            tc.tile_pool(name="z1ps", bufs=2, space="PSUM") as z1ps,
            tc.tile_pool(name="h2ps", bufs=2, space="PSUM") as h2ps,
            tc.tile_pool(name="poolps", bufs=2, space="PSUM") as poolps,
            tc.tile_pool(name="tailps", bufs=2, space="PSUM") as tailps,
        ):
            def cload(name, shape, dram, dt=F32):
                t = cpool.tile(shape, dt, tag=name)
                nc.sync.dma_start(out=t[:], in_=dram[:])
                return t

            w1a = cload("w1a", [STATE_DIM, HID + 1], w1a_d, F32R)
            w2a = cload("w2a", [HID + 1, HID], w2a_d)
            w3a = cload("w3a", [HID + 1, HID], w3a_d, F32R)
            rw1 = cload("rw1", [HID, HID], rw1_d, F32R)
            rw2 = cload("rw2", [HID, HID], rw2_d, F32R)
            rw3 = cload("rw3", [HID, OUT_DIM], rw3_d, F32R)
            pb1a = cload("pb1a", [HID + 1, 1], pb1a_d)
            rb1 = cload("rb1", [HID, 1], rb1_d)
            rb2 = cload("rb2", [HID, 1], rb2_d)
            rb3 = cload("rb3", [OUT_DIM, 1], rb3_d)
            idr = cload("idr", [128, N_WIN * T], idr_d)

            ident = cpool.tile([128, 128], F32, tag="ident")
            make_identity(nc, ident[:])
            iota_i = cpool.tile([128, GRP * 128], I32, tag="iota_i")
            nc.gpsimd.iota(
                iota_i[:], pattern=[[0, GRP], [1, 128]], base=0, channel_multiplier=0
            )
            iota4 = cpool.tile([128, GRP * 128], F32, tag="iota4")
            nc.vector.tensor_copy(out=iota4[:], in_=iota_i[:])

            for _rep in range(reps):
             for ch in range(SEG_PER_CORE // CHUNK):
                poolT = chpool.tile([HID + 1, CHUNK], F32R, tag="poolT")
                nc.sync.dma_start(
                    out=poolT[HID : HID + 1, :],
                    in_=cnt_d[:, ch * CHUNK : (ch + 1) * CHUNK],
                )
                for wl in range(CHUNK // WIN_SEGS):
                    w = ch * (CHUNK // WIN_SEGS) + wl
                    xw = xpool.tile([STATE_DIM, PW], F32R, tag="xw")
                    nc.sync.dma_start(out=xw[:], in_=xT_d[:, w * PW : (w + 1) * PW])

                    pooled_ps = poolps.tile([WIN_SEGS, HID], F32, tag="pool")

                    for g in range(NG):
                        gcols = slice(g * GRP * 128, (g + 1) * GRP * 128)
                        z1_ps = z1ps.tile([HID + 1, GRP * 128], F32, tag="z1")
                        nc.tensor.matmul(
                            out=z1_ps[:], lhsT=w1a[:], rhs=xw[:, gcols],
                            start=True, stop=True,
                        )
                        h1a = wpool.tile([HID + 1, GRP * 128], F32, tag="h1a")
                        nc.scalar.activation(
                            out=h1a[:], in_=z1_ps[:], func=AF.Relu, bias=pb1a[:]
                        )

                        h2_ps = h2ps.tile([128, GRP * HID], F32, tag="h2")
                        for t in range(GRP):
                            nc.tensor.matmul(
                                out=h2_ps[:, t * HID : (t + 1) * HID],
                                lhsT=h1a[:, t * 128 : (t + 1) * 128],
                                rhs=w2a[:],
                                start=True,
                                stop=True,
                            )
                        h2t = wpool.tile([128, GRP * HID], F32, tag="h2t")
                        nc.vector.tensor_scalar(
                            out=h2t[:], in0=h2_ps[:], scalar1=0.0, scalar2=None,
                            op0=mybir.AluOpType.max,
                        )

                        onehot = wpool.tile([128, GRP * 128], F32, tag="onehot")
                        c0 = w * T + g * GRP
                        nc.vector.tensor_tensor(
                            out=onehot[:].rearrange("p (a b) -> p a b", b=128),
                            in0=idr[:, c0 : c0 + GRP].to_broadcast([128, GRP, 128]),
                            in1=iota4[:].rearrange("p (a b) -> p a b", b=128),
                            op=mybir.AluOpType.is_equal,
                        )
                        for t in range(GRP):
                            nc.tensor.matmul(
                                out=pooled_ps[:],
                                lhsT=onehot[:, t * 128 : (t + 1) * 128],
                                rhs=h2t[:, t * HID : (t + 1) * HID],
                                start=(g == 0 and t == 0),
                                stop=(g == NG - 1 and t == GRP - 1),
                            )

                    pooled_sb = wpool.tile([WIN_SEGS, HID], F32, tag="pooled")
                    nc.vector.tensor_copy(out=pooled_sb[:], in_=pooled_ps[:])
                    poolT_ps = tailps.tile([HID, WIN_SEGS], F32, tag="tail")
                    nc.tensor.transpose(
                        out=poolT_ps[:], in_=pooled_sb[:], identity=ident[:]
                    )
                    nc.vector.tensor_copy(
                        out=poolT[:HID, wl * WIN_SEGS : (wl + 1) * WIN_SEGS],
                        in_=poolT_ps[:],
                    )

                # batched phi-L3 + rho tail over this 512-seg chunk
                p3_ps = tailps.tile([HID, CHUNK], F32, tag="tail")
                nc.tensor.matmul(
                    out=p3_ps[:], lhsT=w3a[:], rhs=poolT[:], start=True, stop=True
                )
                p3 = chpool.tile([HID, CHUNK], F32R, tag="p3")
                nc.scalar.activation(out=p3[:], in_=p3_ps[:], func=AF.Copy, bias=0.0)

                r1_ps = tailps.tile([HID, CHUNK], F32, tag="tail")
                nc.tensor.matmul(
                    out=r1_ps[:], lhsT=rw1[:], rhs=p3[:], start=True, stop=True
                )
                r1 = chpool.tile([HID, CHUNK], F32R, tag="r1")
                nc.scalar.activation(out=r1[:], in_=r1_ps[:], func=AF.Relu, bias=rb1[:])

                r2_ps = tailps.tile([HID, CHUNK], F32, tag="tail")
                nc.tensor.matmul(
                    out=r2_ps[:], lhsT=rw2[:], rhs=r1[:], start=True, stop=True
                )
                r2 = chpool.tile([HID, CHUNK], F32R, tag="r2")
                nc.scalar.activation(out=r2[:], in_=r2_ps[:], func=AF.Relu, bias=rb2[:])

                r3_ps = tailps.tile([OUT_DIM, CHUNK], F32, tag="tail")
                nc.tensor.matmul(
                    out=r3_ps[:], lhsT=rw3[:], rhs=r2[:], start=True, stop=True
                )
                out_sb = chpool.tile([OUT_DIM, CHUNK], F32, tag="outc")
                nc.vector.tensor_scalar(
                    out=out_sb[:], in0=r3_ps[:], scalar1=rb3[:], scalar2=None,
                    op0=mybir.AluOpType.add,
                )
                nc.sync.dma_start(
                    out=out_d[:, ch * CHUNK : (ch + 1) * CHUNK], in_=out_sb[:]
                )

    nc.compile()
    _BUILD_CACHE[(T, reps)] = nc
    return nc


def _prep_inputs(neighbors: np.ndarray, segment_ids: np.ndarray):
    """Shard rows by 128-segment windows; pad each window to T 128-row tiles."""
    x = np.asarray(neighbors, dtype=np.float32)
    ids = np.asarray(segment_ids, dtype=np.int64)
    n_gwin = NUM_SEGMENTS // WIN_SEGS
    edges = np.searchsorted(ids, np.arange(0, NUM_SEGMENTS + 1, WIN_SEGS))
    wcnt = np.diff(edges)
    T = max(GRP, GRP * int(np.ceil(wcnt.max() / (128 * GRP))))
    PW = T * 128

    xT = np.zeros((N_CORES, STATE_DIM, N_WIN * PW), dtype=np.float32)
    idr = np.full((N_CORES, 128, N_WIN * T), -1.0, dtype=np.float32)
    counts = np.bincount(ids, minlength=NUM_SEGMENTS).astype(np.float32)
    cnt = counts.reshape(N_CORES, 1, SEG_PER_CORE)
    for g in range(n_gwin):
        c, wl = divmod(g, N_WIN)
        r0, r1 = int(edges[g]), int(edges[g + 1])
        n = r1 - r0
        if n == 0:
            continue
        base = wl * PW
        xT[c, :, base : base + n] = x[r0:r1].T
        rel = np.full(PW, -1.0, dtype=np.float32)
        rel[:n] = (ids[r0:r1] - g * WIN_SEGS).astype(np.float32)
        idr[c, :, wl * T : (wl + 1) * T] = rel.reshape(T, 128).T
    return xT, idr, cnt, T


def prep_maps(inputs: dict):
    """Host-side marshalling: returns (T, in_maps per core)."""
    xT, idr, cnt, T = _prep_inputs(inputs["neighbors"], inputs["segment_ids"])
    f = lambda a: np.ascontiguousarray(np.asarray(a, dtype=np.float32))
    col = lambda a: f(a).reshape(-1, 1)
    w1a = np.concatenate([f(inputs["phi_W1"]), np.zeros((STATE_DIM, 1), np.float32)], 1)
    pb1a = np.concatenate([col(inputs["phi_b1"]), np.ones((1, 1), np.float32)], 0)
    w2a = np.vstack([f(inputs["phi_W2"]), f(inputs["phi_b2"]).reshape(1, -1)])
    w3a = np.vstack([f(inputs["phi_W3"]), f(inputs["phi_b3"]).reshape(1, -1)])
    shared = {
        "w1a": w1a,
        "w2a": w2a,
        "w3a": w3a,
        "rw1": f(inputs["rho_W1"]),
        "rw2": f(inputs["rho_W2"]),
        "rw3": f(inputs["rho_W3"]),
        "pb1a": pb1a,
        "rb1": col(inputs["rho_b1"]),
        "rb2": col(inputs["rho_b2"]),
        "rb3": col(inputs["rho_b3"]),
    }
    in_maps = [
        {"xT": xT[c], "idr": idr[c], "cnt": cnt[c], **shared} for c in range(N_CORES)
    ]
    return T, in_maps


def kernel(**inputs):
    T, in_maps = prep_maps(inputs)
    nc = _build_program(T)
    res = run_bass_kernel_spmd(nc, in_maps, core_ids=list(range(N_CORES)))
    out = np.concatenate(
        [res.results[c]["out"].T for c in range(N_CORES)], axis=0
    ).astype(np.float32)
    return out
